# revision 44
# baseline (speedup 1.0000x reference)
"""Trainium2 Bass kernel for windowed ViT attention with decomposed relative
position bias (B=8, N=1024=32x32, C=768, 12 heads, head_dim 64).

Sharding: data-parallel over batch B across 8 NeuronCores (1 image per core).

Per-core algorithm (bf16 operands, fp32 PSUM accumulation):
  - q/k computed directly in transposed layout qT/kT [d, n] (and v in
    natural layout [n, d]) from host-pretransposed bf16 x and weights;
    q-scale folded into the q rows of the qkv weight on the host.
  - rel-pos bias folded into the attention matmul by augmenting the
    contraction dim from 64 to exactly 128:
       S_T[k2, q] = sum_d kT[d,k2] qT[d,q]
                  + sum_i Ih[i,k2] rel_hT[i,q] + sum_j Iw[j,k2] rel_wT[j,q]
    where Ih/Iw are constant 0/1 indicator rows and rel_hT/rel_wT come from
    one small matmul per h (resp. w) against a row-flipped rel table
    (Toeplitz slicing), batched over all heads.  PE cost is N-columns-bound,
    so the bias rides for free.
  - softmax denominator rides as a ones-row appended to V (M=65); exp is
    fused with PSUM evacuation on the scalar engine (the ACT exp stream is
    the phase-D floor at ~100us).
  - phase D chunk loop is software-pipelined: S_T(ch+1) issues before
    attn@V(ch) so the in-order PE never waits for exp(ch); the next pair's
    k-projection matmuls/copies are spread one-per-chunk through odd heads.
  - softmax reciprocal: heads 0-9 via the gpsimd DRAM-roundtrip broadcast
    (3-stage pipeline, off critical path); heads 10-11 via a PE one-row
    broadcast matmul (e65 indicator lhsT) to kill the end-of-D tail.
  - phase B fuses the v projection into the ACT-bound rel-pos section and
    round-robins the rel evac copies over ACT/DVE.
  - attn output is produced in [c, n] layout feeding the final projection
    as the stationary operand, producing natural [n, c] output.
"""

import sys

if "/opt/trn_rl_repo" not in sys.path:
    sys.path.insert(0, "/opt/trn_rl_repo")

import numpy as np

NUM_HEADS = 12
N_CTX = 1024
C_DIM = 768
HD = 64
HH = 32
NCORES = 8

_CACHE: dict = {}


def _build_nc(reps=1):
    import concourse.mybir as mybir
    import concourse.tile as tile
    from concourse import bacc
    from contextlib import ExitStack

    f32 = mybir.dt.float32
    f32r = mybir.dt.float32r
    bf16 = mybir.dt.bfloat16
    Exp = mybir.ActivationFunctionType.Exp

    nc = bacc.Bacc("TRN2", target_bir_lowering=False, debug=False)

    def mm(out, lhsT, rhs, **kw):
        nc.tensor.matmul(out, lhsT, rhs, **kw)

    xt = nc.dram_tensor("xt", [768, 1024], bf16, kind="ExternalInput").ap()
    wqk = nc.dram_tensor("wqk", [768, 1536], bf16, kind="ExternalInput").ap()
    wv = nc.dram_tensor("wv", [768, 768], bf16, kind="ExternalInput").ap()
    wp = nc.dram_tensor("wp", [768, 768], bf16, kind="ExternalInput").ap()
    bias = nc.dram_tensor("bias", [128, 768], f32, kind="ExternalInput").ap()
    ind = nc.dram_tensor("ind", [64, 1024], bf16, kind="ExternalInput").ap()
    rfh = nc.dram_tensor("rfh", [64, 63], bf16, kind="ExternalInput").ap()
    rfw = nc.dram_tensor("rfw", [64, 63], bf16, kind="ExternalInput").ap()
    y = nc.dram_tensor("y", [1024, 768], f32, kind="ExternalOutput").ap()

    with tile.TileContext(nc) as tc, ExitStack() as es:
        singles = es.enter_context(tc.tile_pool(name="singles", bufs=1))
        dram = es.enter_context(tc.tile_pool(name="dram", bufs=1, space="DRAM"))

        # qaug: per head a [128, 1024] aug-rhs block: rows 0:64 = qT (scaled),
        # 64:96 = rel_hT, 96:128 = rel_wT. Heads side by side in columns.
        qaug = singles.tile([128, 12 * 1024], bf16)
        # v in natural layout + ones column per head: [k2-part, chunk, head, 65]
        vaug = singles.tile([128, 8, 12, 65], bf16)
        rfh_sb = singles.tile([64, 63], bf16)
        rfw_sb = singles.tile([64, 63], bf16)
        # e65: one-hot row-64 selector; broadcasts the denominator row of the
        # [65, n] unnorm tile across 64 partitions via a single matmul.
        e65 = singles.tile([65, 64], bf16)
        # Assembled S_T lhsT tiles: rows 0:64 = kT chunk, rows 64:128 =
        # constant indicator rows. Indexed [pair-parity][head-parity][chunk].
        kasm = [
            [
                [singles.tile([128, 128], bf16, name=f"kasm{b}_{p}_{c}") for c in range(8)]
                for p in range(2)
            ]
            for b in range(2)
        ]

        nc.gpsimd.dma_start(rfh_sb, rfh)
        nc.gpsimd.dma_start(rfw_sb, rfw)
        nc.vector.memset(e65, 0.0)
        nc.vector.memset(e65[64:65, :], 1.0)

        for _rep in range(reps):
          with ExitStack() as esR:
            den_dram = dram.tile([12, 1024], f32)
            rec_dram = dram.tile([12, 1024], f32)

            xtp = esR.enter_context(tc.tile_pool(name="xtp", bufs=1))
            late = esR.enter_context(tc.tile_pool(name="late", bufs=1))
            xt_sb = xtp.tile([128, 6, 1024], bf16)
            # attn out, [c, n] layout; one tile per head pair so phase E's
            # per-k matmuls only depend on the pair they actually read
            atile = [
                late.tile([128, 1024], bf16, name=f"atile{t}") for t in range(6)
            ]

            xt_r = xt.rearrange("(ko p) n -> p ko n", p=128)
            wqk_r = wqk.rearrange("(ko p) n -> p ko n", p=128)
            wv_r = wv.rearrange("(ko p) n -> p ko n", p=128)

            # ------- Phase B/C: q projection, rel-pos rows, v projection -------
            with ExitStack() as esB:
                bigB = esB.enter_context(tc.tile_pool(name="bigB", bufs=1))
                wq_sb = bigB.tile([128, 6, 768], bf16)
                wv_sb = bigB.tile([128, 6, 768], bf16)
                ind_sb = bigB.tile([64, 1024], bf16)
                nc.gpsimd.dma_start(ind_sb, ind)
                for b in range(2):
                    for p in range(2):
                        for c in range(8):
                            nc.vector.tensor_copy(
                                kasm[b][p][c][64:128, :],
                                ind_sb[:, c * 128 : (c + 1) * 128],
                            )
                # warm the exp table set during the DMA head
                warm = bigB.tile([1, 1], f32)
                nc.vector.memset(warm, 0.0)
                nc.scalar.activation(warm, warm, Exp)
                # wq in column halves: the first 6 q-proj PSUM groups (m<3)
                # need only cols 0:384 of each k-chunk, so PE starts sooner.
                for k in range(6):
                    nc.sync.dma_start(xt_sb[:, k], xt_r[:, k])
                    nc.sync.dma_start(wq_sb[:, k, 0:384], wqk_r[:, k, 0:384])
                for k in range(6):
                    nc.sync.dma_start(wq_sb[:, k, 384:768], wqk_r[:, k, 384:768])
                wk0 = bigB.tile([128, 6, 128], bf16)
                for k in range(6):
                    nc.sync.dma_start(wk0[:, k], wqk_r[:, k, 768:896])
                for k in range(6):
                    nc.sync.dma_start(wv_sb[:, k], wv_r[:, k])

                # q, transposed layout: out rows = head*64+d, cols = n
                with ExitStack() as esQ:
                    bqk = esQ.enter_context(
                        tc.tile_pool(name="bqk", bufs=2, space="PSUM")
                    )
                    for m in range(6):
                        for n in range(2):
                            ps = bqk.tile([128, 512], f32, name="qp")
                            for k in range(6):
                                mm(
                                    ps,
                                    wq_sb[:, k, m * 128 : (m + 1) * 128],
                                    xt_sb[:, k, n * 512 : (n + 1) * 512],
                                    start=(k == 0),
                                    stop=(k == 5),
                                )
                            for half, hd in ((0, 2 * m), (64, 2 * m + 1)):
                                nc.scalar.copy(
                                    qaug[0:64, hd * 1024 + n * 512 : hd * 1024 + (n + 1) * 512],
                                    ps[half : half + 64, :],
                                )
                    # pair-0 k projection rides in the q pool; its kasm
                    # copies drain on DVE under the rel section.
                    for n in range(2):
                        kp0 = bqk.tile([128, 512], f32, name="qp")
                        for k in range(6):
                            mm(
                                kp0,
                                wk0[:, k],
                                xt_sb[:, k, n * 512 : (n + 1) * 512],
                                start=(k == 0),
                                stop=(k == 5),
                            )
                        for c in range(4):
                            ch = n * 4 + c
                            eng0 = nc.scalar.copy if c % 2 == 0 else nc.vector.tensor_copy
                            eng1 = nc.vector.tensor_copy if c % 2 == 0 else nc.scalar.copy
                            eng0(
                                kasm[0][0][ch][0:64, :],
                                kp0[0:64, c * 128 : (c + 1) * 128],
                            )
                            eng1(
                                kasm[0][1][ch][0:64, :],
                                kp0[64:128, c * 128 : (c + 1) * 128],
                            )

                # rel/v PSUM pools allocate after the q scope closes so they
                # reuse its banks (max 8 concurrent PSUM banks).
                bv = esB.enter_context(tc.tile_pool(name="bv", bufs=1, space="PSUM"))
                cps = esB.enter_context(tc.tile_pool(name="cps", bufs=3, space="PSUM"))

                # Fused: rel-pos rows interleaved with the v projection.
                # rel_hT[k,(head,h,w)] = sum_c rel_pos_h[h-k+31,c] *
                # qT[c,(head,h,w)]; one matmul per h (w) over all heads via
                # the flipped-table slice.  The v chunks ride in the PE slack
                # while ACT/DVE drain the rel evac copies.
                qaug4d = qaug.rearrange("p (hd a b) -> p hd a b", hd=12, a=32)

                def v_chunk(ch):
                    pv = bv.tile([128, 768], f32, name="pv")
                    for c0, cw in ((0, 512), (512, 256)):
                        for k in range(6):
                            mm(
                                pv[:, c0 : c0 + cw],
                                xt_sb[:, k, ch * 128 : (ch + 1) * 128],
                                wv_sb[:, k, c0 : c0 + cw],
                                start=(k == 0),
                                stop=(k == 5),
                            )
                    nc.vector.tensor_copy(
                        vaug[:, ch, :, 0:64], pv.rearrange("p (h d) -> p h d", h=12)
                    )
                    nc.vector.memset(vaug[:, ch, :, 64:65], 1.0)

                # Two rel slices share one [32, 1024] PSUM tile (one 384-col
                # matmul per bank half), evacuated by a single 768-free copy
                # — halving the per-copy overhead that saturates ACT/DVE.
                for j in range(16):
                    pgh = cps.tile([32, 2, 512], f32, name="pg")
                    pgw = cps.tile([32, 2, 512], f32, name="pg")
                    for i2 in range(2):
                        i = 2 * j + i2
                        mm(pgh[:, i2, 0:384], rfh_sb[:, 31 - i : 63 - i],
                           qaug4d[0:64, :, i, :], start=True, stop=True)
                        mm(pgw[:, i2, 0:384], rfw_sb[:, 31 - i : 63 - i],
                           qaug4d[0:64, :, :, i], start=True, stop=True)
                    srch = pgh[:, :, 0:384].rearrange("p i (hd w) -> p i hd w", hd=12)
                    dsth = qaug4d[64:96, :, 2 * j : 2 * j + 2, :].rearrange(
                        "p hd i w -> p i hd w"
                    )
                    srcw = pgw[:, :, 0:384].rearrange("p i (hd h) -> p i hd h", hd=12)
                    dstw = qaug4d[96:128, :, :, 2 * j : 2 * j + 2].rearrange(
                        "p hd h i -> p i hd h"
                    )
                    if j % 2 == 0:
                        nc.scalar.copy(dsth, srch)
                        nc.vector.tensor_copy(dstw, srcw)
                    else:
                        nc.vector.tensor_copy(dsth, srch)
                        nc.scalar.copy(dstw, srcw)
                    if j % 2 == 1:
                        v_chunk(j // 2)

            # ---------------- Phase D: attention per head ----------------
            wpsp = esR.enter_context(tc.tile_pool(name="wpsp", bufs=1))
            wp_sb = wpsp.tile([128, 6, 768], bf16)
            bias_sb = wpsp.tile([128, 768], f32)

            esD = esR.enter_context(ExitStack())
            expp = esD.enter_context(tc.tile_pool(name="expp", bufs=3))
            smalls = esD.enter_context(tc.tile_pool(name="smalls", bufs=2))
            unp = esD.enter_context(tc.tile_pool(name="unp", bufs=3))
            wkp = esD.enter_context(tc.tile_pool(name="wkp", bufs=2))
            dps = esD.enter_context(tc.tile_pool(name="dps", bufs=2, space="PSUM"))
            dpo = esD.enter_context(tc.tile_pool(name="dpo", bufs=2, space="PSUM"))

            wkt_t = {}

            def fetch_wk(t):
                wkt = wkp.tile([128, 6, 128], bf16)
                wkt_t[t] = wkt
                for k in range(6):
                    nc.sync.dma_start(
                        wkt[:, k], wqk_r[:, k, 768 + t * 128 : 768 + (t + 1) * 128]
                    )

            kp_t = {}

            def mk_mm(t, n, k):
                def f():
                    if k == 0:
                        kp_t[(t, n)] = dpo.tile([128, 512], f32, name=f"ops{n}")
                    mm(
                        kp_t[(t, n)],
                        wkt_t[t][:, k],
                        xt_sb[:, k, n * 512 : (n + 1) * 512],
                        start=(k == 0),
                        stop=(k == 5),
                    )
                return f

            def mk_cp(t, n, c):
                def f():
                    ch = n * 4 + c
                    kp = kp_t[(t, n)]
                    nc.vector.tensor_copy(
                        kasm[t % 2][0][ch][0:64, :],
                        kp[0:64, c * 128 : (c + 1) * 128],
                    )
                    nc.vector.tensor_copy(
                        kasm[t % 2][1][ch][0:64, :],
                        kp[64:128, c * 128 : (c + 1) * 128],
                    )
                    if (n, c) == (1, 3):
                        kp_t.pop((t, 0))
                        kp_t.pop((t, 1))
                        wkt_t.pop(t)
                return f

            def k_slots_even(t):
                # pair t's k-projection n=0 half: one matmul per chunk 2-7 of
                # the pair's EVEN predecessor head, so the PE load balances
                # against the ACT exp stream instead of doubling up on the
                # odd head.  (Chunks 0-1 would stall on the o_ps ring.)
                slots = [[] for _ in range(8)]
                for k in range(6):
                    slots[2 + k].append(mk_mm(t, 0, k))
                return slots

            def k_slots_odd(t):
                # n=1 matmuls in chunks 0-5, n=0 copies in 1-2, n=1 copies
                # in 6-7 of the odd predecessor head.
                slots = [[] for _ in range(8)]
                for k in range(6):
                    slots[k].append(mk_mm(t, 1, k))
                for i, c in enumerate([0, 1, 2, 3]):
                    slots[1 + i // 2].append(mk_cp(t, 0, c))
                for i, c in enumerate([0, 1, 2, 3]):
                    slots[6 + i // 2].append(mk_cp(t, 1, c))
                return slots

            fetch_wk(1)

            o_ps_h = {}
            d128_h = {}
            rep_h = {}
            un_h = {}

            def tail_norm(hd, un):
                # PE-broadcast softmax normalization for the last two heads:
                # den row -> 64 partitions via e65 matmul, reciprocal on DVE,
                # then normalize straight into atile.
                den_ps = dps.tile([128, 1024], f32, tag="sps")
                for nt in range(2):
                    mm(
                        den_ps[0:64, nt * 512 : (nt + 1) * 512],
                        e65,
                        un[:, nt * 512 : (nt + 1) * 512],
                        start=True,
                        stop=True,
                    )
                t3 = hd // 2
                half3 = (hd % 2) * 64
                # half-pipelined: ACT-evac'd un halves flow through mm ->
                # reciprocal -> normalize per 512-col half
                for nt in range(2):
                    sl = slice(nt * 512, (nt + 1) * 512)
                    rep = smalls.tile([64, 512], f32, name="rep2")
                    nc.vector.reciprocal(rep, den_ps[0:64, sl])
                    nc.vector.tensor_mul(
                        atile[t3][half3 : half3 + 64, sl], un[0:64, sl], rep
                    )

            # pend: attn@V calls deferred two chunks behind their S_T/exp so
            # the in-order PE never idles waiting on the ACT exp stream; the
            # queue carries across heads.  When a head's last chunk drains,
            # finish_head evacuates its o_ps (the old stage-1).
            pend = []

            def finish_head(hd):
                o_ps = o_ps_h.pop(hd)
                # rows 0:64 = unnormalized out, row 64 = denominator.
                # heads 10/11 evac to bf16 so the e65 broadcast matmul gets a
                # properly-rounded operand (f32->f32r bitcasts are rejected).
                if hd >= 10:
                    un = unp.tile([65, 1024], bf16, name="un16")
                else:
                    un = unp.tile([65, 1024], f32, name="unnorm")
                un_h[hd] = un
                for nt in range(2):
                    if hd == 11:
                        # ACT is idle after the last exp; evac there so the
                        # tail chain (mm -> recip -> mul) pipelines per half
                        nc.scalar.copy(un[:, nt * 512 : (nt + 1) * 512], o_ps[nt])
                    else:
                        nc.vector.tensor_copy(
                            un[:, nt * 512 : (nt + 1) * 512], o_ps[nt]
                        )
                if hd < 10:
                    d128 = smalls.tile([128, 8], f32, name="d128")
                    d128_h[hd] = d128
                    nc.gpsimd.dma_start(d128, un[64:65, :])

            def drain_one():
                ex, ch, hd, o_ps = pend.pop(0)
                for nt in range(2):
                    mm(
                        o_ps[nt],
                        vaug[:, ch, hd, :],
                        ex[:, nt * 512 : (nt + 1) * 512],
                        start=(ch == 0),
                        stop=(ch == 7),
                    )
                if ch == 7:
                    finish_head(hd)

            for it in range(13):
                # stage 2 (head=it-2): reciprocal + broadcast via DRAM
                if 0 <= it - 2 < 10:
                    hd = it - 2
                    d128 = d128_h.pop(hd)
                    r128 = smalls.tile([128, 8], f32, name="r128")
                    nc.vector.reciprocal(r128, d128)
                    nc.gpsimd.dma_start(
                        rec_dram[hd].rearrange("(p f) -> p f", p=128), r128
                    )
                    rep = smalls.tile([64, 1024], f32, name="rep")
                    rep_h[hd] = rep
                    nc.gpsimd.dma_start(
                        rep, rec_dram[hd : hd + 1, :].to_broadcast((64, 1024))
                    )

                # stage 3 (head=it-3): normalize into atile
                if 0 <= it - 3 < 10:
                    hd = it - 3
                    t3 = hd // 2
                    half3 = (hd % 2) * 64
                    rep = rep_h.pop(hd)
                    un = un_h.pop(hd)
                    nc.vector.tensor_mul(
                        atile[t3][half3 : half3 + 64, :], un[0:64, :], rep
                    )

                if it == 6:
                    nc.sync.dma_start(bias_sb, bias)
                    wp_r = wp.rearrange("(ko p) n -> p ko n", p=128)
                    for k in range(6):
                        nc.sync.dma_start(wp_sb[:, k], wp_r[:, k])

                # stage 0 (head=it): software-pipelined attention chunks.
                # PE issue order: S(0), S(1), A(0), S(2), A(1), ... so the
                # in-order PE never waits on exp(ch); the next pair's k
                # projection is spread one slot per chunk.
                if it < 12:
                    hd = it
                    par = hd % 2
                    t = hd // 2
                    slots = [[] for _ in range(8)]
                    if t + 1 < 6:
                        slots = (k_slots_even if par == 0 else k_slots_odd)(t + 1)
                        if par == 1 and t + 2 < 6:
                            fetch_wk(t + 2)
                    o_ps = [dpo.tile([65, 512], f32, name=f"ops{nt}") for nt in range(2)]
                    o_ps_h[hd] = o_ps

                    for ch in range(8):
                        s_ps = dps.tile([128, 1024], f32, tag="sps")
                        for nt in range(2):
                            mm(
                                s_ps[:, nt * 512 : (nt + 1) * 512],
                                kasm[t % 2][par][ch],
                                qaug[:, hd * 1024 + nt * 512 : hd * 1024 + (nt + 1) * 512],
                                start=True,
                                stop=True,
                            )
                        ex = expp.tile([128, 1024], bf16)
                        nc.scalar.activation(ex, s_ps, Exp)
                        if len(pend) >= 2:
                            drain_one()
                        for w in slots[ch]:
                            w()
                        # head 10's tail normalization rides in the slack of
                        # head 11's last chunk (den_ps reuses s_ps(6)'s slot);
                        # drain one extra attn@V so only (7,11) trails the loop
                        if hd == 11 and ch == 7:
                            drain_one()
                            tail_norm(10, un_h.pop(10))
                        pend.append((ex, ch, hd, o_ps))

                    if hd == 11:
                        while pend:
                            drain_one()
                        tail_norm(11, un_h.pop(11))

            # ---------------- Phase E: output projection ----------------
            esD.close()
            with ExitStack() as esE:
                eps = esE.enter_context(tc.tile_pool(name="eps", bufs=2, space="PSUM"))
                epool = esE.enter_context(tc.tile_pool(name="epool", bufs=3))
                for ch in range(8):
                    yp = eps.tile([128, 768], f32)
                    for c0, cw in ((0, 512), (512, 256)):
                        for k in range(6):
                            mm(
                                yp[:, c0 : c0 + cw],
                                atile[k][:, ch * 128 : (ch + 1) * 128],
                                wp_sb[:, k, c0 : c0 + cw],
                                start=(k == 0),
                                stop=(k == 5),
                            )
                    y_sb = epool.tile([128, 768], f32)
                    if ch < 7:
                        nc.vector.tensor_add(y_sb, yp, bias_sb)
                        nc.sync.dma_start(y[ch * 128 : (ch + 1) * 128, :], y_sb)
                    else:
                        # split the last chunk so the final DMA overlaps the
                        # final bias add instead of serializing after it
                        for c0 in (0, 384):
                            nc.vector.tensor_add(
                                y_sb[:, c0 : c0 + 384],
                                yp[:, c0 : c0 + 384],
                                bias_sb[:, c0 : c0 + 384],
                            )
                            nc.sync.dma_start(
                                y[ch * 128 : (ch + 1) * 128, c0 : c0 + 384],
                                y_sb[:, c0 : c0 + 384],
                            )

    nc.compile()
    return nc


def _host_prep(qkv_w, rel_pos_h, rel_pos_w, proj_w, proj_b):
    import ml_dtypes

    bf = ml_dtypes.bfloat16
    qkv_w = np.asarray(qkv_w, np.float32)
    scale = 1.0 / np.sqrt(HD)
    wqk = np.ascontiguousarray(qkv_w[0:1536].T)  # [768, 1536]
    wqk[:, 0:768] *= scale
    wv = np.ascontiguousarray(qkv_w[1536:2304].T)  # [768, 768]
    wp = np.ascontiguousarray(np.asarray(proj_w, np.float32).T)  # [768, 768]
    bias = np.ascontiguousarray(
        np.broadcast_to(np.asarray(proj_b, np.float32)[None, :], (128, 768))
    )
    k2 = np.arange(1024)
    indm = np.zeros((64, 1024), np.float32)
    indm[0:32] = (k2[None, :] // 32) == np.arange(32)[:, None]
    indm[32:64] = (k2[None, :] % 32) == np.arange(32)[:, None]
    rfh = np.ascontiguousarray(np.asarray(rel_pos_h, np.float32)[::-1].T)  # [64, 63]
    rfw = np.ascontiguousarray(np.asarray(rel_pos_w, np.float32)[::-1].T)
    return dict(
        wqk=wqk.astype(bf),
        wv=wv.astype(bf),
        wp=wp.astype(bf),
        bias=bias,
        ind=indm.astype(bf),
        rfh=rfh.astype(bf),
        rfw=rfw.astype(bf),
    )


def get_nc(reps=1):
    key = ("nc", reps)
    if key not in _CACHE:
        _CACHE[key] = _build_nc(reps=reps)
    return _CACHE[key]


def make_in_maps(x, qkv_w, rel_pos_h, rel_pos_w, proj_w, proj_b):
    import ml_dtypes

    shared = _host_prep(qkv_w, rel_pos_h, rel_pos_w, proj_w, proj_b)
    x = np.asarray(x, np.float32)
    return [
        dict(
            shared,
            xt=np.ascontiguousarray(x[b].T).astype(ml_dtypes.bfloat16),
        )
        for b in range(x.shape[0])
    ]


def kernel(x, qkv_w, rel_pos_h, rel_pos_w, proj_w, proj_b, H=32, W=32):
    from concourse.bass_utils import run_bass_kernel_spmd

    nc = get_nc()
    in_maps = make_in_maps(x, qkv_w, rel_pos_h, rel_pos_w, proj_w, proj_b)
    res = run_bass_kernel_spmd(nc, in_maps, list(range(NCORES)))
    out = np.stack([np.asarray(res.results[b]["y"]) for b in range(NCORES)])
    return out.astype(np.float32)


# revision 45
# speedup vs baseline: 1.1548x; 1.1548x over previous
"""Trainium2 Bass kernel for windowed ViT attention with decomposed relative
position bias (B=8, N=1024=32x32, C=768, 12 heads, head_dim 64).

Sharding: data-parallel over batch B across 8 NeuronCores (1 image per core).

Per-core algorithm (bf16 operands, fp32 PSUM accumulation):
  - q/k computed directly in transposed layout qT/kT [d, n] (and v in
    natural layout [n, d]) from host-pretransposed bf16 x and weights;
    q-scale folded into the q rows of the qkv weight on the host.
  - rel-pos bias folded into the attention matmul by augmenting the
    contraction dim from 64 to exactly 128:
       S_T[k2, q] = sum_d kT[d,k2] qT[d,q]
                  + sum_i Ih[i,k2] rel_hT[i,q] + sum_j Iw[j,k2] rel_wT[j,q]
    where Ih/Iw are constant 0/1 indicator rows and rel_hT/rel_wT come from
    one small matmul per h (resp. w) against a row-flipped rel table
    (Toeplitz slicing), batched over all heads.  PE cost is N-columns-bound,
    so the bias rides for free.
  - softmax denominator rides as a ones-row appended to V (M=65); exp is
    fused with PSUM evacuation on the scalar engine (the ACT exp stream is
    the phase-D floor at ~100us).
  - phase D chunk loop is software-pipelined: S_T(ch+1) issues before
    attn@V(ch) so the in-order PE never waits for exp(ch); the next pair's
    k-projection matmuls/copies are spread one-per-chunk through odd heads.
  - softmax reciprocal: heads 0-9 via the gpsimd DRAM-roundtrip broadcast
    (3-stage pipeline, off critical path); heads 10-11 via a PE one-row
    broadcast matmul (e65 indicator lhsT) to kill the end-of-D tail.
  - phase B fuses the v projection into the ACT-bound rel-pos section and
    round-robins the rel evac copies over ACT/DVE.
  - attn output is produced in [c, n] layout feeding the final projection
    as the stationary operand, producing natural [n, c] output.
"""

import sys

if "/opt/trn_rl_repo" not in sys.path:
    sys.path.insert(0, "/opt/trn_rl_repo")

import numpy as np

NUM_HEADS = 12
N_CTX = 1024
C_DIM = 768
HD = 64
HH = 32
NCORES = 8

_CACHE: dict = {}


def _build_nc(reps=1):
    import concourse.mybir as mybir
    import concourse.tile as tile
    from concourse import bacc
    from contextlib import ExitStack

    f32 = mybir.dt.float32
    f32r = mybir.dt.float32r
    bf16 = mybir.dt.bfloat16
    Exp = mybir.ActivationFunctionType.Exp

    nc = bacc.Bacc("TRN2", target_bir_lowering=False, debug=False)

    def mm(out, lhsT, rhs, **kw):
        nc.tensor.matmul(out, lhsT, rhs, **kw)

    xt = nc.dram_tensor("xt", [768, 1024], bf16, kind="ExternalInput").ap()
    wqk = nc.dram_tensor("wqk", [768, 1536], bf16, kind="ExternalInput").ap()
    wv = nc.dram_tensor("wv", [768, 768], bf16, kind="ExternalInput").ap()
    wp = nc.dram_tensor("wp", [768, 768], bf16, kind="ExternalInput").ap()
    bias = nc.dram_tensor("bias", [128, 768], f32, kind="ExternalInput").ap()
    ind = nc.dram_tensor("ind", [64, 1024], f32r, kind="ExternalInput").ap()
    rfh = nc.dram_tensor("rfh", [64, 63], f32r, kind="ExternalInput").ap()
    rfw = nc.dram_tensor("rfw", [64, 63], f32r, kind="ExternalInput").ap()
    y = nc.dram_tensor("y", [1024, 768], f32, kind="ExternalOutput").ap()

    with tile.TileContext(nc) as tc, ExitStack() as es:
        singles = es.enter_context(tc.tile_pool(name="singles", bufs=1))
        dram = es.enter_context(tc.tile_pool(name="dram", bufs=1, space="DRAM"))

        # qaug: per head a [128, 1024] aug-rhs block: rows 0:64 = qT (scaled),
        # 64:96 = rel_hT, 96:128 = rel_wT. Heads side by side in columns.
        qaug = singles.tile([128, 12 * 1024], f32r)
        # v in natural layout + ones column per head: [k2-part, chunk, head, 65]
        vaug = singles.tile([128, 8, 12, 65], f32r)
        rfh_sb = singles.tile([64, 63], f32r)
        rfw_sb = singles.tile([64, 63], f32r)
        # e65: one-hot row-64 selector; broadcasts the denominator row of the
        # [65, n] unnorm tile across 64 partitions via a single matmul.
        e65 = singles.tile([65, 64], bf16)
        # Assembled S_T lhsT tiles: rows 0:64 = kT chunk, rows 64:128 =
        # constant indicator rows. Indexed [pair-parity][head-parity][chunk].
        kasm = [
            [
                [singles.tile([128, 128], f32r, name=f"kasm{b}_{p}_{c}") for c in range(8)]
                for p in range(2)
            ]
            for b in range(2)
        ]

        nc.gpsimd.dma_start(rfh_sb, rfh)
        nc.gpsimd.dma_start(rfw_sb, rfw)
        nc.vector.memset(e65, 0.0)
        nc.vector.memset(e65[64:65, :], 1.0)

        for _rep in range(reps):
          with ExitStack() as esR:
            den_dram = dram.tile([12, 1024], f32)
            rec_dram = dram.tile([12, 1024], f32)

            xtp = esR.enter_context(tc.tile_pool(name="xtp", bufs=1))
            late = esR.enter_context(tc.tile_pool(name="late", bufs=1))
            xt_sb = xtp.tile([128, 6, 1024], bf16)
            # attn out, [c, n] layout; one tile per head pair so phase E's
            # per-k matmuls only depend on the pair they actually read
            atile = [
                late.tile([128, 1024], bf16, name=f"atile{t}") for t in range(6)
            ]

            xt_r = xt.rearrange("(ko p) n -> p ko n", p=128)
            wqk_r = wqk.rearrange("(ko p) n -> p ko n", p=128)
            wv_r = wv.rearrange("(ko p) n -> p ko n", p=128)

            # ------- Phase B/C: q projection, rel-pos rows, v projection -------
            with ExitStack() as esB:
                bigB = esB.enter_context(tc.tile_pool(name="bigB", bufs=1))
                wq_sb = bigB.tile([128, 6, 768], bf16)
                wv_sb = bigB.tile([128, 6, 768], bf16)
                ind_sb = bigB.tile([64, 1024], f32r)
                nc.gpsimd.dma_start(ind_sb, ind)
                for b in range(2):
                    for p in range(2):
                        for c in range(8):
                            nc.vector.tensor_copy(
                                kasm[b][p][c][64:128, :],
                                ind_sb[:, c * 128 : (c + 1) * 128],
                            )
                # warm the exp table set during the DMA head
                warm = bigB.tile([1, 1], f32)
                nc.vector.memset(warm, 0.0)
                nc.scalar.activation(warm, warm, Exp)
                # wq in column halves: the first 6 q-proj PSUM groups (m<3)
                # need only cols 0:384 of each k-chunk, so PE starts sooner.
                for k in range(6):
                    nc.sync.dma_start(xt_sb[:, k], xt_r[:, k])
                    nc.sync.dma_start(wq_sb[:, k, 0:384], wqk_r[:, k, 0:384])
                for k in range(6):
                    nc.sync.dma_start(wq_sb[:, k, 384:768], wqk_r[:, k, 384:768])
                wk0 = bigB.tile([128, 6, 128], bf16)
                for k in range(6):
                    nc.sync.dma_start(wk0[:, k], wqk_r[:, k, 768:896])
                for k in range(6):
                    nc.sync.dma_start(wv_sb[:, k], wv_r[:, k])

                # q, transposed layout: out rows = head*64+d, cols = n
                with ExitStack() as esQ:
                    bqk = esQ.enter_context(
                        tc.tile_pool(name="bqk", bufs=2, space="PSUM")
                    )
                    for m in range(6):
                        for n in range(2):
                            ps = bqk.tile([128, 512], f32, name="qp")
                            for k in range(6):
                                mm(
                                    ps,
                                    wq_sb[:, k, m * 128 : (m + 1) * 128],
                                    xt_sb[:, k, n * 512 : (n + 1) * 512],
                                    start=(k == 0),
                                    stop=(k == 5),
                                )
                            for half, hd in ((0, 2 * m), (64, 2 * m + 1)):
                                nc.scalar.copy(
                                    qaug[0:64, hd * 1024 + n * 512 : hd * 1024 + (n + 1) * 512],
                                    ps[half : half + 64, :],
                                )
                    # pair-0 k projection rides in the q pool; its kasm
                    # copies drain on DVE under the rel section.
                    for n in range(2):
                        kp0 = bqk.tile([128, 512], f32, name="qp")
                        for k in range(6):
                            mm(
                                kp0,
                                wk0[:, k],
                                xt_sb[:, k, n * 512 : (n + 1) * 512],
                                start=(k == 0),
                                stop=(k == 5),
                            )
                        for c in range(4):
                            ch = n * 4 + c
                            eng0 = nc.scalar.copy if c % 2 == 0 else nc.vector.tensor_copy
                            eng1 = nc.vector.tensor_copy if c % 2 == 0 else nc.scalar.copy
                            eng0(
                                kasm[0][0][ch][0:64, :],
                                kp0[0:64, c * 128 : (c + 1) * 128],
                            )
                            eng1(
                                kasm[0][1][ch][0:64, :],
                                kp0[64:128, c * 128 : (c + 1) * 128],
                            )

                # rel/v PSUM pools allocate after the q scope closes so they
                # reuse its banks (max 8 concurrent PSUM banks).
                bv = esB.enter_context(tc.tile_pool(name="bv", bufs=1, space="PSUM"))
                cps = esB.enter_context(tc.tile_pool(name="cps", bufs=3, space="PSUM"))

                # Fused: rel-pos rows interleaved with the v projection.
                # rel_hT[k,(head,h,w)] = sum_c rel_pos_h[h-k+31,c] *
                # qT[c,(head,h,w)]; one matmul per h (w) over all heads via
                # the flipped-table slice.  The v chunks ride in the PE slack
                # while ACT/DVE drain the rel evac copies.
                qaug4d = qaug.rearrange("p (hd a b) -> p hd a b", hd=12, a=32)

                def v_chunk(ch):
                    pv = bv.tile([128, 768], f32, name="pv")
                    for c0, cw in ((0, 512), (512, 256)):
                        for k in range(6):
                            mm(
                                pv[:, c0 : c0 + cw],
                                xt_sb[:, k, ch * 128 : (ch + 1) * 128],
                                wv_sb[:, k, c0 : c0 + cw],
                                start=(k == 0),
                                stop=(k == 5),
                            )
                    nc.vector.tensor_copy(
                        vaug[:, ch, :, 0:64], pv.rearrange("p (h d) -> p h d", h=12)
                    )
                    nc.vector.memset(vaug[:, ch, :, 64:65].bitcast(f32), 1.0)

                # Two rel slices share one [32, 1024] PSUM tile (one 384-col
                # matmul per bank half), evacuated by a single 768-free copy
                # — halving the per-copy overhead that saturates ACT/DVE.
                for j in range(16):
                    pgh = cps.tile([32, 2, 512], f32, name="pg")
                    pgw = cps.tile([32, 2, 512], f32, name="pg")
                    for i2 in range(2):
                        i = 2 * j + i2
                        mm(pgh[:, i2, 0:384], rfh_sb[:, 31 - i : 63 - i],
                           qaug4d[0:64, :, i, :], start=True, stop=True)
                        mm(pgw[:, i2, 0:384], rfw_sb[:, 31 - i : 63 - i],
                           qaug4d[0:64, :, :, i], start=True, stop=True)
                    srch = pgh[:, :, 0:384].rearrange("p i (hd w) -> p i hd w", hd=12)
                    dsth = qaug4d[64:96, :, 2 * j : 2 * j + 2, :].rearrange(
                        "p hd i w -> p i hd w"
                    )
                    srcw = pgw[:, :, 0:384].rearrange("p i (hd h) -> p i hd h", hd=12)
                    dstw = qaug4d[96:128, :, :, 2 * j : 2 * j + 2].rearrange(
                        "p hd h i -> p i hd h"
                    )
                    if j % 2 == 0:
                        nc.scalar.copy(dsth, srch)
                        nc.vector.tensor_copy(dstw, srcw)
                    else:
                        nc.vector.tensor_copy(dsth, srch)
                        nc.scalar.copy(dstw, srcw)
                    if j % 2 == 1:
                        v_chunk(j // 2)

            # ---------------- Phase D: attention per head ----------------
            wpsp = esR.enter_context(tc.tile_pool(name="wpsp", bufs=1))
            wp_sb = wpsp.tile([128, 6, 768], bf16)
            bias_sb = wpsp.tile([128, 768], f32)

            esD = esR.enter_context(ExitStack())
            expp = esD.enter_context(tc.tile_pool(name="expp", bufs=3))
            smalls = esD.enter_context(tc.tile_pool(name="smalls", bufs=2))
            unp = esD.enter_context(tc.tile_pool(name="unp", bufs=3))
            wkp = esD.enter_context(tc.tile_pool(name="wkp", bufs=2))
            dps = esD.enter_context(tc.tile_pool(name="dps", bufs=2, space="PSUM"))
            dpo = esD.enter_context(tc.tile_pool(name="dpo", bufs=2, space="PSUM"))

            wkt_t = {}

            def fetch_wk(t):
                wkt = wkp.tile([128, 6, 128], bf16)
                wkt_t[t] = wkt
                for k in range(6):
                    nc.sync.dma_start(
                        wkt[:, k], wqk_r[:, k, 768 + t * 128 : 768 + (t + 1) * 128]
                    )

            kp_t = {}

            def mk_mm(t, n, k):
                def f():
                    if k == 0:
                        kp_t[(t, n)] = dpo.tile([128, 512], f32, name=f"ops{n}")
                    mm(
                        kp_t[(t, n)],
                        wkt_t[t][:, k],
                        xt_sb[:, k, n * 512 : (n + 1) * 512],
                        start=(k == 0),
                        stop=(k == 5),
                    )
                return f

            def mk_cp(t, n, c):
                def f():
                    ch = n * 4 + c
                    kp = kp_t[(t, n)]
                    nc.vector.tensor_copy(
                        kasm[t % 2][0][ch][0:64, :],
                        kp[0:64, c * 128 : (c + 1) * 128],
                    )
                    nc.vector.tensor_copy(
                        kasm[t % 2][1][ch][0:64, :],
                        kp[64:128, c * 128 : (c + 1) * 128],
                    )
                    if (n, c) == (1, 3):
                        kp_t.pop((t, 0))
                        kp_t.pop((t, 1))
                        wkt_t.pop(t)
                return f

            def k_slots_even(t):
                # pair t's k-projection n=0 half: one matmul per chunk 2-7 of
                # the pair's EVEN predecessor head, so the PE load balances
                # against the ACT exp stream instead of doubling up on the
                # odd head.  (Chunks 0-1 would stall on the o_ps ring.)
                slots = [[] for _ in range(8)]
                for k in range(6):
                    slots[2 + k].append(mk_mm(t, 0, k))
                return slots

            def k_slots_odd(t):
                # n=1 matmuls in chunks 0-5, n=0 copies in 1-2, n=1 copies
                # in 6-7 of the odd predecessor head.
                slots = [[] for _ in range(8)]
                for k in range(6):
                    slots[k].append(mk_mm(t, 1, k))
                for i, c in enumerate([0, 1, 2, 3]):
                    slots[1 + i // 2].append(mk_cp(t, 0, c))
                for i, c in enumerate([0, 1, 2, 3]):
                    slots[6 + i // 2].append(mk_cp(t, 1, c))
                return slots

            fetch_wk(1)

            o_ps_h = {}
            d128_h = {}
            rep_h = {}
            un_h = {}

            def tail_norm(hd, un):
                # PE-broadcast softmax normalization for the last two heads:
                # den row -> 64 partitions via e65 matmul, reciprocal on DVE,
                # then normalize straight into atile.
                den_ps = dps.tile([128, 1024], f32, tag="sps")
                for nt in range(2):
                    mm(
                        den_ps[0:64, nt * 512 : (nt + 1) * 512],
                        e65,
                        un[:, nt * 512 : (nt + 1) * 512],
                        start=True,
                        stop=True,
                    )
                t3 = hd // 2
                half3 = (hd % 2) * 64
                # half-pipelined: ACT-evac'd un halves flow through mm ->
                # reciprocal -> normalize per 512-col half
                for nt in range(2):
                    sl = slice(nt * 512, (nt + 1) * 512)
                    rep = smalls.tile([64, 512], f32, name="rep2")
                    nc.vector.reciprocal(rep, den_ps[0:64, sl])
                    nc.vector.tensor_mul(
                        atile[t3][half3 : half3 + 64, sl], un[0:64, sl], rep
                    )

            # pend: attn@V calls deferred two chunks behind their S_T/exp so
            # the in-order PE never idles waiting on the ACT exp stream; the
            # queue carries across heads.  When a head's last chunk drains,
            # finish_head evacuates its o_ps (the old stage-1).
            pend = []

            def finish_head(hd):
                o_ps = o_ps_h.pop(hd)
                # rows 0:64 = unnormalized out, row 64 = denominator.
                # heads 10/11 evac to bf16 so the e65 broadcast matmul gets a
                # properly-rounded operand (f32->f32r bitcasts are rejected).
                if hd >= 10:
                    un = unp.tile([65, 1024], bf16, name="un16")
                else:
                    un = unp.tile([65, 1024], f32, name="unnorm")
                un_h[hd] = un
                for nt in range(2):
                    if hd == 11:
                        # ACT is idle after the last exp; evac there so the
                        # tail chain (mm -> recip -> mul) pipelines per half
                        nc.scalar.copy(un[:, nt * 512 : (nt + 1) * 512], o_ps[nt])
                    else:
                        nc.vector.tensor_copy(
                            un[:, nt * 512 : (nt + 1) * 512], o_ps[nt]
                        )
                if hd < 10:
                    d128 = smalls.tile([128, 8], f32, name="d128")
                    d128_h[hd] = d128
                    nc.gpsimd.dma_start(d128, un[64:65, :])

            def drain_one():
                ex, ch, hd, o_ps = pend.pop(0)
                for nt in range(2):
                    mm(
                        o_ps[nt],
                        vaug[:, ch, hd, :],
                        ex[:, nt * 512 : (nt + 1) * 512],
                        start=(ch == 0),
                        stop=(ch == 7),
                    )
                if ch == 7:
                    finish_head(hd)

            for it in range(13):
                # stage 2 (head=it-2): reciprocal + broadcast via DRAM
                if 0 <= it - 2 < 10:
                    hd = it - 2
                    d128 = d128_h.pop(hd)
                    r128 = smalls.tile([128, 8], f32, name="r128")
                    nc.vector.reciprocal(r128, d128)
                    nc.gpsimd.dma_start(
                        rec_dram[hd].rearrange("(p f) -> p f", p=128), r128
                    )
                    rep = smalls.tile([64, 1024], f32, name="rep")
                    rep_h[hd] = rep
                    nc.gpsimd.dma_start(
                        rep, rec_dram[hd : hd + 1, :].to_broadcast((64, 1024))
                    )

                # stage 3 (head=it-3): normalize into atile
                if 0 <= it - 3 < 10:
                    hd = it - 3
                    t3 = hd // 2
                    half3 = (hd % 2) * 64
                    rep = rep_h.pop(hd)
                    un = un_h.pop(hd)
                    nc.vector.tensor_mul(
                        atile[t3][half3 : half3 + 64, :], un[0:64, :], rep
                    )

                if it == 6:
                    nc.sync.dma_start(bias_sb, bias)
                    wp_r = wp.rearrange("(ko p) n -> p ko n", p=128)
                    for k in range(6):
                        nc.sync.dma_start(wp_sb[:, k], wp_r[:, k])

                # stage 0 (head=it): software-pipelined attention chunks.
                # PE issue order: S(0), S(1), A(0), S(2), A(1), ... so the
                # in-order PE never waits on exp(ch); the next pair's k
                # projection is spread one slot per chunk.
                if it < 12:
                    hd = it
                    par = hd % 2
                    t = hd // 2
                    slots = [[] for _ in range(8)]
                    if t + 1 < 6:
                        slots = (k_slots_even if par == 0 else k_slots_odd)(t + 1)
                        if par == 1 and t + 2 < 6:
                            fetch_wk(t + 2)
                    o_ps = [dpo.tile([65, 512], f32, name=f"ops{nt}") for nt in range(2)]
                    o_ps_h[hd] = o_ps

                    for ch in range(8):
                        s_ps = dps.tile([128, 1024], f32, tag="sps")
                        for nt in range(2):
                            mm(
                                s_ps[:, nt * 512 : (nt + 1) * 512],
                                kasm[t % 2][par][ch],
                                qaug[:, hd * 1024 + nt * 512 : hd * 1024 + (nt + 1) * 512],
                                start=True,
                                stop=True,
                            )
                        ex = expp.tile([128, 1024], f32r)
                        nc.scalar.activation(ex, s_ps, Exp)
                        if len(pend) >= 2:
                            drain_one()
                        for w in slots[ch]:
                            w()
                        # head 10's tail normalization rides in the slack of
                        # head 11's last chunk (den_ps reuses s_ps(6)'s slot);
                        # drain one extra attn@V so only (7,11) trails the loop
                        if hd == 11 and ch == 7:
                            drain_one()
                            tail_norm(10, un_h.pop(10))
                        pend.append((ex, ch, hd, o_ps))

                    if hd == 11:
                        while pend:
                            drain_one()
                        tail_norm(11, un_h.pop(11))

            # ---------------- Phase E: output projection ----------------
            esD.close()
            with ExitStack() as esE:
                eps = esE.enter_context(tc.tile_pool(name="eps", bufs=2, space="PSUM"))
                epool = esE.enter_context(tc.tile_pool(name="epool", bufs=3))
                for ch in range(8):
                    yp = eps.tile([128, 768], f32)
                    for c0, cw in ((0, 512), (512, 256)):
                        for k in range(6):
                            mm(
                                yp[:, c0 : c0 + cw],
                                atile[k][:, ch * 128 : (ch + 1) * 128],
                                wp_sb[:, k, c0 : c0 + cw],
                                start=(k == 0),
                                stop=(k == 5),
                            )
                    y_sb = epool.tile([128, 768], f32)
                    if ch < 7:
                        nc.vector.tensor_add(y_sb, yp, bias_sb)
                        nc.sync.dma_start(y[ch * 128 : (ch + 1) * 128, :], y_sb)
                    else:
                        # split the last chunk so the final DMA overlaps the
                        # final bias add instead of serializing after it
                        for c0 in (0, 384):
                            nc.vector.tensor_add(
                                y_sb[:, c0 : c0 + 384],
                                yp[:, c0 : c0 + 384],
                                bias_sb[:, c0 : c0 + 384],
                            )
                            nc.sync.dma_start(
                                y[ch * 128 : (ch + 1) * 128, c0 : c0 + 384],
                                y_sb[:, c0 : c0 + 384],
                            )

    nc.compile()
    return nc


def _host_prep(qkv_w, rel_pos_h, rel_pos_w, proj_w, proj_b):
    import ml_dtypes

    bf = ml_dtypes.bfloat16
    qkv_w = np.asarray(qkv_w, np.float32)
    scale = 1.0 / np.sqrt(HD)
    wqk = np.ascontiguousarray(qkv_w[0:1536].T)  # [768, 1536]
    wqk[:, 0:768] *= scale
    wv = np.ascontiguousarray(qkv_w[1536:2304].T)  # [768, 768]
    wp = np.ascontiguousarray(np.asarray(proj_w, np.float32).T)  # [768, 768]
    bias = np.ascontiguousarray(
        np.broadcast_to(np.asarray(proj_b, np.float32)[None, :], (128, 768))
    )
    k2 = np.arange(1024)
    indm = np.zeros((64, 1024), np.float32)
    indm[0:32] = (k2[None, :] // 32) == np.arange(32)[:, None]
    indm[32:64] = (k2[None, :] % 32) == np.arange(32)[:, None]
    rfh = np.ascontiguousarray(np.asarray(rel_pos_h, np.float32)[::-1].T)  # [64, 63]
    rfw = np.ascontiguousarray(np.asarray(rel_pos_w, np.float32)[::-1].T)
    return dict(
        wqk=wqk.astype(bf),
        wv=wv.astype(bf),
        wp=wp.astype(bf),
        bias=bias,
        ind=indm,
        rfh=rfh,
        rfw=rfw,
    )


def get_nc(reps=1):
    key = ("nc", reps)
    if key not in _CACHE:
        _CACHE[key] = _build_nc(reps=reps)
    return _CACHE[key]


def make_in_maps(x, qkv_w, rel_pos_h, rel_pos_w, proj_w, proj_b):
    import ml_dtypes

    shared = _host_prep(qkv_w, rel_pos_h, rel_pos_w, proj_w, proj_b)
    x = np.asarray(x, np.float32)
    return [
        dict(
            shared,
            xt=np.ascontiguousarray(x[b].T).astype(ml_dtypes.bfloat16),
        )
        for b in range(x.shape[0])
    ]


def kernel(x, qkv_w, rel_pos_h, rel_pos_w, proj_w, proj_b, H=32, W=32):
    from concourse.bass_utils import run_bass_kernel_spmd

    nc = get_nc()
    in_maps = make_in_maps(x, qkv_w, rel_pos_h, rel_pos_w, proj_w, proj_b)
    res = run_bass_kernel_spmd(nc, in_maps, list(range(NCORES)))
    out = np.stack([np.asarray(res.results[b]["y"]) for b in range(NCORES)])
    return out.astype(np.float32)
